# revision 2
# baseline (speedup 1.0000x reference)
"""HCNN (known-U) recurrence kernel for 8 Trainium2 NeuronCores.

Model (see reference): 80 sequential steps of
    state' = tanh(cat(post_state, u)) @ A            A: (2112, 2048) fp32
with teacher forcing post_state[:, :128] = y during the 64 past steps,
outputs = 64 past errors then 16 forecasts (first 128 state components).

Strategy
--------
Data-parallel over batch: 256 = 8 cores x 32. Each core runs the full
recurrence for its batch slice; no collectives.

Per-core per-step matmul x @ A with batch M=32 would waste 3/4 of the
128-wide PE array, so the A columns are split into 4 interleaved groups
and computed by 4 concurrent column-tiled matmuls (tile_position=(0,32j))
sharing the array. Data is fp16 (single pass): the teacher-forced
recurrence is strongly contractive, emulation shows end-to-end output
error ~1.5e-4 relative vs the fp32 reference.

Column interleave: state column s lives in col-group j=(s//32)%4 at free
offset 32*(s//128) + s%32. With that mapping the (128, 512) psum holding
state' (batch on partitions within each 32-group) turns into the next
step's stationary operand layout via a single DVE 32x32 block-transpose:
block (j, m') lands at partitions [32j:32j+32] of k-tile m' -- exactly
where matmul round m' reads it. ACT applies tanh (psum -> fp16 SBUF),
DVE transposes, PE consumes; y/u/init contributions are pre-tanh'ed and
pre-transposed on the host, so past-step rounds k=0 (y) and k=16 (u)
have no dependency on the transpose and hide its latency.
"""

import sys

for _p in ("/opt/trn_rl_repo", "/root/.axon_site/_ro/trn_rl_repo"):
    if _p not in sys.path:
        sys.path.insert(0, _p)

import numpy as np

N_STATE = 2048
N_U = 64
N_Y = 128
PAST = 64
FORE = 16
BATCH = 256
T = PAST + FORE          # 80 total steps; only 79 matmul steps needed
NSTEP = T - 1            # step t computes state_{t+1}; state_80 is unused
NK = 17                  # contraction tiles: 16 x 128 state + 1 x (64 u + 64 pad)
KDIM = NK * 128          # 2176 padded contraction size
N_CORES = 8
B = BATCH // N_CORES     # 32 per core


def _build_program():
    import concourse.bass as bass
    import concourse.tile as tile
    from concourse import mybir

    F32 = mybir.dt.float32
    F16 = mybir.dt.float16

    nc = bass.Bass("TRN2", target_bir_lowering=False, debug=False,
                   num_devices=N_CORES)

    A_ext = nc.declare_dram_parameter("A_re", [KDIM, 4, 512], F16, isOutput=False)
    ytanhT_ext = nc.declare_dram_parameter("ytanhT", [128, PAST * B], F16, isOutput=False)
    utanhT_ext = nc.declare_dram_parameter("utanhT", [128, NSTEP * B], F16, isOutput=False)
    ywrap_ext = nc.declare_dram_parameter("ywrap", [128, (PAST - 1) * B], F32, isOutput=False)
    initxT_ext = nc.declare_dram_parameter("initxT", [128, 512], F16, isOutput=False)
    out_ext = nc.declare_dram_parameter("outbuf", [128, NSTEP * B], F32, isOutput=True)

    with tile.TileContext(nc) as tc:
        with tc.tile_pool(name="const", bufs=1) as cpool, \
             tc.tile_pool(name="xbuf", bufs=2) as xpool, \
             tc.tile_pool(name="th", bufs=2) as thpool, \
             tc.tile_pool(name="psum", bufs=2, space="PSUM") as pspool:

            # DMA order = first-use order: init x strips + y (feed step 0's
            # first rounds), then A tiles in round emission order so step 0
            # can begin as soon as the first tiles land. ywrap is only read
            # by the DVE output path, so it loads last.
            A_sb = cpool.tile([128, NK * 2048], F16, tag="A")
            ytanhT = cpool.tile([128, PAST * B], F16, tag="yt")
            utanhT = cpool.tile([128, NSTEP * B], F16, tag="ut")
            ywrap = cpool.tile([128, (PAST - 1) * B], F32, tag="yw")
            outbuf = cpool.tile([128, NSTEP * B], F32, tag="ob")
            xlo = xpool.tile([128, 256], F16, tag="xlo")
            xhi = xpool.tile([128, 256], F16, tag="xhi")

            nc.sync.dma_start(out=xlo[:], in_=initxT_ext[:, 0:256])
            nc.sync.dma_start(out=xhi[:], in_=initxT_ext[:, 256:512])
            nc.sync.dma_start(out=ytanhT[:], in_=ytanhT_ext[:])
            korder = [0, 16] + list(range(1, 16))
            for k in korder:
                if k == 0:
                    nc.sync.dma_start(out=A_sb[:, 0:2048], in_=A_ext[0:128, :, :])
                elif k == 16:
                    nc.sync.dma_start(out=utanhT[:], in_=utanhT_ext[:])
                    nc.sync.dma_start(out=A_sb[:, 2048 * 16:2048 * 17],
                                      in_=A_ext[128 * 16:128 * 17, :, :])
                else:
                    nc.sync.dma_start(out=A_sb[:, 2048 * k:2048 * (k + 1)],
                                      in_=A_ext[128 * k:128 * (k + 1), :, :])
            nc.sync.dma_start(out=ywrap[:], in_=ywrap_ext[:])

            def lhs_for(t, k, lo, hi):
                if k == 0:
                    if t < PAST:
                        return ytanhT[:, B * t:B * (t + 1)]
                    return lo[:, 0:32]
                if k == 16:
                    return utanhT[:, B * t:B * (t + 1)]
                if k < 8:
                    return lo[:, 32 * k:32 * (k + 1)]
                return hi[:, 32 * (k - 8):32 * (k - 7)]

            for t in range(NSTEP):
                ps = pspool.tile([128, 512], F32, tag="ps")
                last = t == NSTEP - 1
                # Each k-round is split into lo (psum cols 0:256) and hi
                # (256:512) halves, all lo rounds emitted first: the lo half
                # of the accumulation finishes ~2us before the hi half, so
                # tanh+transpose of the lo strips overlap the hi matmuls and
                # the next step's k=8..15 rounds see their operand in time.
                # The final step only needs psum cols 0:32 (the forecast).
                halves = ((0, 32),) if last else ((0, 256), (256, 512))
                for h0, h1 in halves:
                    for idx, k in enumerate(korder):
                        lhsT = lhs_for(t, k, xlo, xhi)
                        start = idx == 0
                        stop = idx == len(korder) - 1
                        for j in range(4):
                            nc.tensor.matmul(
                                ps[32 * j:32 * (j + 1), h0:h1],
                                lhsT,
                                A_sb[:, 2048 * k + 512 * j + h0:
                                     2048 * k + 512 * j + h1],
                                start=start, stop=stop,
                                tile_position=(0, 32 * j),
                            )

                if not last:
                    th_lo = thpool.tile([128, 256], F16, tag="thlo")
                    nc.scalar.activation(th_lo[:], ps[:, 0:256],
                                         mybir.ActivationFunctionType.Tanh)
                    nlo = xpool.tile([128, 256], F16, tag="xlo")
                    nc.vector.transpose(nlo[:], th_lo[:])
                    th_hi = thpool.tile([128, 256], F16, tag="thhi")
                    nc.scalar.activation(th_hi[:], ps[:, 256:512],
                                         mybir.ActivationFunctionType.Tanh)
                    nhi = xpool.tile([128, 256], F16, tag="xhi")
                    nc.vector.transpose(nhi[:], th_hi[:])

                # output slot t+1 from this psum (expectation = cols 0:128 of
                # state', living in psum[:, 0:32] across all partition groups);
                # emitted after the transposes so the DVE unblocks them first.
                if t + 1 < PAST:
                    nc.vector.tensor_sub(outbuf[:, B * t:B * (t + 1)],
                                         ps[:, 0:32],
                                         ywrap[:, B * t:B * (t + 1)])
                else:
                    nc.vector.tensor_copy(outbuf[:, B * t:B * (t + 1)],
                                          ps[:, 0:32])

                if not last:
                    xlo, xhi = nlo, nhi

                # stream finished output slots out while compute continues
                if t % 16 == 15:
                    nc.sync.dma_start(out=out_ext[:, B * (t - 15):B * (t + 1)],
                                      in_=outbuf[:, B * (t - 15):B * (t + 1)])

            done = (NSTEP // 16) * 16
            nc.sync.dma_start(out=out_ext[:, B * done:],
                              in_=outbuf[:, B * done:])

    _split_multi_waits(nc)
    return nc


def _split_multi_waits(nc):
    """This walrus build accepts at most one sem wait per instruction; Tile
    sometimes emits more. Hoist extras onto nops inserted just before the
    instruction in the same engine stream."""
    from concourse import mybir

    n = 0
    for f in nc.m.functions:
        for b in f.blocks:
            insts = b.instructions
            out = []
            changed = False
            for ins in insts:
                si = ins.sync_info
                if si is not None and len(si.on_wait) > 1:
                    waits = list(si.on_wait)
                    for w in waits[:-1]:
                        n += 1
                        out.append(mybir.InstNoOp(
                            name=f"I-waitsplit-{n}",
                            engine=ins.engine,
                            ins=[], outs=[],
                            bass_nofuse=True,
                            sync_info=mybir.SyncInfo(on_wait=[w], on_update=[]),
                        ))
                    ins.sync_info = mybir.SyncInfo(
                        on_wait=[waits[-1]], on_update=list(si.on_update))
                    changed = True
                out.append(ins)
            if changed:
                b.instructions = out


def _host_inputs(U, Y, A, init_state):
    """Build the per-core input maps (all pre-tanh / pre-transpose work)."""
    A = np.asarray(A, np.float32)
    U = np.asarray(U, np.float32)
    Y = np.asarray(Y, np.float32)
    init_state = np.asarray(init_state, np.float32)

    A_pad = np.zeros((KDIM, N_STATE), np.float16)
    A_pad[:N_STATE + N_U] = A.astype(np.float16)
    # column interleave: col s -> (j=(s//32)%4, free 32*(s//128)+s%32)
    A_re = np.ascontiguousarray(
        A_pad.reshape(KDIM, 16, 4, 32).transpose(0, 2, 1, 3).reshape(KDIM, 4, 512))

    init_tanh = np.tanh(init_state[0]).astype(np.float16)          # (2048,)
    initxT = np.ascontiguousarray(
        np.broadcast_to(init_tanh.reshape(16, 128).T[:, None, :].transpose(0, 2, 1),
                        (128, 16, 32)).reshape(128, 512))

    ytanh = np.tanh(Y).astype(np.float16)                          # (64, 256, 128)
    utanh = np.tanh(U[:NSTEP]).astype(np.float16)                  # (79, 256, 64)

    in_maps = []
    for c in range(N_CORES):
        b0 = c * B
        yt = np.ascontiguousarray(
            ytanh[:, b0:b0 + B, :].transpose(0, 2, 1)              # (64, 128, 32)
            .transpose(1, 0, 2).reshape(128, PAST * B))
        ut = np.zeros((128, NSTEP * B), np.float16)
        ut[:N_U] = (utanh[:, b0:b0 + B, :].transpose(0, 2, 1)      # (79, 64, 32)
                    .transpose(1, 0, 2).reshape(N_U, NSTEP * B))
        # ywrap slot s (=1..63) at cols 32*(s-1): rows 32j+b = Y[s, b0+b, 32j+cc]
        yw = (Y[1:PAST, b0:b0 + B, :].reshape(PAST - 1, B, 4, 32)
              .transpose(0, 2, 1, 3)                               # (63, 4, 32b, 32cc)
              .reshape(PAST - 1, 128, 32)
              .transpose(1, 0, 2).reshape(128, (PAST - 1) * B))
        in_maps.append({
            "A_re": A_re,
            "ytanhT": yt,
            "utanhT": np.ascontiguousarray(ut),
            "ywrap": np.ascontiguousarray(yw.astype(np.float32)),
            "initxT": initxT,
        })
    return in_maps


def kernel(U, Y, A, init_state):
    from concourse.bass_utils import run_bass_kernel_spmd

    nc = _build_program()
    in_maps = _host_inputs(U, Y, A, init_state)
    res = run_bass_kernel_spmd(nc, in_maps, list(range(N_CORES)))

    out = np.empty((T, BATCH, N_Y), np.float32)
    # slot 0: err for t=0 is pure host math (state_0 = broadcast init_state)
    out[0] = np.asarray(init_state, np.float32)[0, :N_Y][None, :] - np.asarray(Y, np.float32)[0]
    for c in range(N_CORES):
        b0 = c * B
        ob = res.results[c]["outbuf"]                              # (128, 79*32)
        # [32j+b, 32t+cc] = out[t+1, b0+b, 32j+cc]
        ob4 = ob.reshape(4, 32, NSTEP, 32)                         # (j, b, t, cc)
        out[1:, b0:b0 + B, :] = ob4.transpose(2, 1, 0, 3).reshape(NSTEP, B, N_Y)
    return out


if __name__ == "__main__":
    rng = np.random.default_rng(0)
    U = rng.standard_normal((T, BATCH, N_U)).astype(np.float32)
    Y = rng.standard_normal((PAST, BATCH, N_Y)).astype(np.float32)
    A = (rng.standard_normal((N_STATE + N_U, N_STATE)) * 0.02).astype(np.float32)
    init = rng.standard_normal((1, N_STATE)).astype(np.float32)
    o = kernel(U=U, Y=Y, A=A, init_state=init)
    print("kernel out:", o.shape, o.dtype)



# revision 4
# speedup vs baseline: 1.0717x; 1.0717x over previous
"""HCNN (known-U) recurrence kernel for 8 Trainium2 NeuronCores.

Model (see reference): 80 sequential steps of
    state' = tanh(cat(post_state, u)) @ A            A: (2112, 2048) fp32
with teacher forcing post_state[:, :128] = y during the 64 past steps,
outputs = 64 past errors then 16 forecasts (first 128 state components).

Strategy
--------
Data-parallel over batch: 256 = 8 cores x 32. Each core runs the full
recurrence for its batch slice; no collectives.

Per-core per-step matmul x @ A with batch M=32 would waste 3/4 of the
128-wide PE array, so the A columns are split into 4 interleaved groups
and computed by 4 concurrent column-tiled matmuls (tile_position=(0,32j))
sharing the array. Data is fp16 (single pass): the teacher-forced
recurrence is strongly contractive, emulation shows end-to-end output
error ~1.5e-4 relative vs the fp32 reference.

Column interleave: state column s lives in col-group j=(s//32)%4 at free
offset 32*(s//128) + s%32. With that mapping the (128, 512) psum holding
state' (batch on partitions within each 32-group) turns into the next
step's stationary operand layout via a single DVE 32x32 block-transpose:
block (j, m') lands at partitions [32j:32j+32] of k-tile m' -- exactly
where matmul round m' reads it. ACT applies tanh (psum -> fp16 SBUF),
DVE transposes, PE consumes; y/u/init contributions are pre-tanh'ed and
pre-transposed on the host, so past-step rounds k=0 (y) and k=16 (u)
have no dependency on the transpose and hide its latency.
"""

import sys

for _p in ("/opt/trn_rl_repo", "/root/.axon_site/_ro/trn_rl_repo"):
    if _p not in sys.path:
        sys.path.insert(0, _p)

import numpy as np

N_STATE = 2048
N_U = 64
N_Y = 128
PAST = 64
FORE = 16
BATCH = 256
T = PAST + FORE          # 80 total steps; only 79 matmul steps needed
NSTEP = T - 1            # step t computes state_{t+1}; state_80 is unused
NK = 17                  # contraction tiles: 16 x 128 state + 1 x (64 u + 64 pad)
KDIM = NK * 128          # 2176 padded contraction size
N_CORES = 8
B = BATCH // N_CORES     # 32 per core


def _build_program():
    import concourse.bass as bass
    import concourse.tile as tile
    from concourse import mybir

    F32 = mybir.dt.float32
    F16 = mybir.dt.float16

    nc = bass.Bass("TRN2", target_bir_lowering=False, debug=False,
                   num_devices=N_CORES)

    A_ext = nc.declare_dram_parameter("A_re", [KDIM, 4, 512], F16, isOutput=False)
    ytanhT_ext = nc.declare_dram_parameter("ytanhT", [128, PAST * B], F16, isOutput=False)
    utanhT_ext = nc.declare_dram_parameter("utanhT", [128, NSTEP * B], F16, isOutput=False)
    ywrap_ext = nc.declare_dram_parameter("ywrap", [128, (PAST - 1) * B], F32, isOutput=False)
    initxT_ext = nc.declare_dram_parameter("initxT", [128, 512], F16, isOutput=False)
    out_ext = nc.declare_dram_parameter("outbuf", [128, NSTEP * B], F32, isOutput=True)

    with tile.TileContext(nc) as tc:
        with tc.tile_pool(name="const", bufs=1) as cpool, \
             tc.tile_pool(name="xbuf", bufs=2) as xpool, \
             tc.tile_pool(name="th", bufs=2) as thpool, \
             tc.tile_pool(name="psum", bufs=2, space="PSUM") as pspool:

            # DMA order = first-use order: init x strips + y (feed step 0's
            # first rounds), then A tiles in round emission order so step 0
            # can begin as soon as the first tiles land. ywrap is only read
            # by the DVE output path, so it loads last.
            A_sb = cpool.tile([128, NK * 2048], F16, tag="A")
            ytanhT = cpool.tile([128, PAST * B], F16, tag="yt")
            utanhT = cpool.tile([128, NSTEP * B], F16, tag="ut")
            ywrap = cpool.tile([128, (PAST - 1) * B], F32, tag="yw")
            outbuf = cpool.tile([128, NSTEP * B], F32, tag="ob")
            xlo = xpool.tile([128, 256], F16, tag="xlo")
            xhi = xpool.tile([128, 256], F16, tag="xhi")

            # critical first-use loads on the sync ring; the bulk A tiles
            # split across both HWDGE rings (sync + scalar) so issue and
            # transfer bandwidth add up.
            nc.sync.dma_start(out=ytanhT[:], in_=ytanhT_ext[:])
            nc.sync.dma_start(out=A_sb[:, 0:2048], in_=A_ext[0:128, :, :])
            nc.scalar.dma_start(out=xlo[:], in_=initxT_ext[:, 0:256])
            nc.scalar.dma_start(out=xhi[:], in_=initxT_ext[:, 256:512])
            nc.sync.dma_start(out=utanhT[:], in_=utanhT_ext[:])
            nc.sync.dma_start(out=A_sb[:, 2048 * 16:2048 * 17],
                              in_=A_ext[128 * 16:128 * 17, :, :])
            korder = [0, 16] + list(range(1, 16))
            for i, k in enumerate(range(1, 16)):
                eng = nc.sync if i % 2 == 0 else nc.scalar
                eng.dma_start(out=A_sb[:, 2048 * k:2048 * (k + 1)],
                              in_=A_ext[128 * k:128 * (k + 1), :, :])
            nc.scalar.dma_start(out=ywrap[:], in_=ywrap_ext[:])

            def lhs_for(t, k, lo, hi):
                if k == 0:
                    if t < PAST:
                        return ytanhT[:, B * t:B * (t + 1)]
                    return lo[:, 0:32]
                if k == 16:
                    return utanhT[:, B * t:B * (t + 1)]
                if k < 8:
                    return lo[:, 32 * k:32 * (k + 1)]
                return hi[:, 32 * (k - 8):32 * (k - 7)]

            for t in range(NSTEP):
                last = t == NSTEP - 1
                # Each k-round is split into a lo half (state cols 0:1024 ->
                # psum cols 0:256) and a hi half, accumulating into two
                # DIFFERENT psum banks (tiles padded to a full 2KB bank).
                # All lo rounds are emitted first: the lo bank closes ~2us
                # before the hi bank, so tanh+transpose of the lo strips run
                # on ACT/DVE while the PE streams the hi matmuls (legal only
                # across banks: PE-write + ACT-read of one bank is fatal and
                # Tile would serialize it). The hi tanh/transpose are split
                # in two 128-col chunks so next step's k=8..11 rounds unlock
                # after the first chunk. The final step only needs psum cols
                # 0:32 (the forecast output).
                ps_lo = pspool.tile([128, 512], F32, tag="pslo")
                wlo = 32 if last else 256
                for idx, k in enumerate(korder):
                    lhsT = lhs_for(t, k, xlo, xhi)
                    for j in range(4):
                        nc.tensor.matmul(
                            ps_lo[32 * j:32 * (j + 1), 0:wlo],
                            lhsT,
                            A_sb[:, 2048 * k + 512 * j:
                                 2048 * k + 512 * j + wlo],
                            start=idx == 0, stop=idx == len(korder) - 1,
                            tile_position=(0, 32 * j),
                        )
                if not last:
                    ps_hi = pspool.tile([128, 512], F32, tag="pshi")
                    for idx, k in enumerate(korder):
                        lhsT = lhs_for(t, k, xlo, xhi)
                        for j in range(4):
                            nc.tensor.matmul(
                                ps_hi[32 * j:32 * (j + 1), 0:256],
                                lhsT,
                                A_sb[:, 2048 * k + 512 * j + 256:
                                     2048 * k + 512 * j + 512],
                                start=idx == 0, stop=idx == len(korder) - 1,
                                tile_position=(0, 32 * j),
                            )

                    th_lo = thpool.tile([128, 256], F16, tag="thlo")
                    nc.scalar.activation(th_lo[:], ps_lo[:, 0:256],
                                         mybir.ActivationFunctionType.Tanh)
                    nlo = xpool.tile([128, 256], F16, tag="xlo")
                    nc.vector.transpose(nlo[:], th_lo[:])
                    th_hi = thpool.tile([128, 256], F16, tag="thhi")
                    nhi = xpool.tile([128, 256], F16, tag="xhi")
                    for c0 in (0, 128):
                        nc.scalar.activation(th_hi[:, c0:c0 + 128],
                                             ps_hi[:, c0:c0 + 128],
                                             mybir.ActivationFunctionType.Tanh)
                        nc.vector.transpose(nhi[:, c0:c0 + 128],
                                            th_hi[:, c0:c0 + 128])

                # output slot t+1 (expectation = cols 0:128 of state', living
                # in ps_lo[:, 0:32] across all partition groups); emitted
                # after the transposes so the DVE unblocks them first.
                if t + 1 < PAST:
                    nc.vector.tensor_sub(outbuf[:, B * t:B * (t + 1)],
                                         ps_lo[:, 0:32],
                                         ywrap[:, B * t:B * (t + 1)])
                else:
                    nc.vector.tensor_copy(outbuf[:, B * t:B * (t + 1)],
                                          ps_lo[:, 0:32])

                if not last:
                    xlo, xhi = nlo, nhi

                # stream finished output slots out while compute continues
                if t % 16 == 15:
                    nc.sync.dma_start(out=out_ext[:, B * (t - 15):B * (t + 1)],
                                      in_=outbuf[:, B * (t - 15):B * (t + 1)])

            done = (NSTEP // 16) * 16
            nc.sync.dma_start(out=out_ext[:, B * done:],
                              in_=outbuf[:, B * done:])

    _split_multi_waits(nc)
    return nc


def _split_multi_waits(nc):
    """This walrus build accepts at most one sem wait per instruction; Tile
    sometimes emits more. Hoist extras onto nops inserted just before the
    instruction in the same engine stream."""
    from concourse import mybir

    n = 0
    for f in nc.m.functions:
        for b in f.blocks:
            insts = b.instructions
            out = []
            changed = False
            for ins in insts:
                si = ins.sync_info
                if si is not None and len(si.on_wait) > 1:
                    waits = list(si.on_wait)
                    for w in waits[:-1]:
                        n += 1
                        out.append(mybir.InstNoOp(
                            name=f"I-waitsplit-{n}",
                            engine=ins.engine,
                            ins=[], outs=[],
                            bass_nofuse=True,
                            sync_info=mybir.SyncInfo(on_wait=[w], on_update=[]),
                        ))
                    ins.sync_info = mybir.SyncInfo(
                        on_wait=[waits[-1]], on_update=list(si.on_update))
                    changed = True
                out.append(ins)
            if changed:
                b.instructions = out


def _host_inputs(U, Y, A, init_state):
    """Build the per-core input maps (all pre-tanh / pre-transpose work)."""
    A = np.asarray(A, np.float32)
    U = np.asarray(U, np.float32)
    Y = np.asarray(Y, np.float32)
    init_state = np.asarray(init_state, np.float32)

    A_pad = np.zeros((KDIM, N_STATE), np.float16)
    A_pad[:N_STATE + N_U] = A.astype(np.float16)
    # column interleave: col s -> (j=(s//32)%4, free 32*(s//128)+s%32)
    A_re = np.ascontiguousarray(
        A_pad.reshape(KDIM, 16, 4, 32).transpose(0, 2, 1, 3).reshape(KDIM, 4, 512))

    init_tanh = np.tanh(init_state[0]).astype(np.float16)          # (2048,)
    initxT = np.ascontiguousarray(
        np.broadcast_to(init_tanh.reshape(16, 128).T[:, None, :].transpose(0, 2, 1),
                        (128, 16, 32)).reshape(128, 512))

    ytanh = np.tanh(Y).astype(np.float16)                          # (64, 256, 128)
    utanh = np.tanh(U[:NSTEP]).astype(np.float16)                  # (79, 256, 64)

    in_maps = []
    for c in range(N_CORES):
        b0 = c * B
        yt = np.ascontiguousarray(
            ytanh[:, b0:b0 + B, :].transpose(0, 2, 1)              # (64, 128, 32)
            .transpose(1, 0, 2).reshape(128, PAST * B))
        ut = np.zeros((128, NSTEP * B), np.float16)
        ut[:N_U] = (utanh[:, b0:b0 + B, :].transpose(0, 2, 1)      # (79, 64, 32)
                    .transpose(1, 0, 2).reshape(N_U, NSTEP * B))
        # ywrap slot s (=1..63) at cols 32*(s-1): rows 32j+b = Y[s, b0+b, 32j+cc]
        yw = (Y[1:PAST, b0:b0 + B, :].reshape(PAST - 1, B, 4, 32)
              .transpose(0, 2, 1, 3)                               # (63, 4, 32b, 32cc)
              .reshape(PAST - 1, 128, 32)
              .transpose(1, 0, 2).reshape(128, (PAST - 1) * B))
        in_maps.append({
            "A_re": A_re,
            "ytanhT": yt,
            "utanhT": np.ascontiguousarray(ut),
            "ywrap": np.ascontiguousarray(yw.astype(np.float32)),
            "initxT": initxT,
        })
    return in_maps


def kernel(U, Y, A, init_state):
    from concourse.bass_utils import run_bass_kernel_spmd

    nc = _build_program()
    in_maps = _host_inputs(U, Y, A, init_state)
    res = run_bass_kernel_spmd(nc, in_maps, list(range(N_CORES)))

    out = np.empty((T, BATCH, N_Y), np.float32)
    # slot 0: err for t=0 is pure host math (state_0 = broadcast init_state)
    out[0] = np.asarray(init_state, np.float32)[0, :N_Y][None, :] - np.asarray(Y, np.float32)[0]
    for c in range(N_CORES):
        b0 = c * B
        ob = res.results[c]["outbuf"]                              # (128, 79*32)
        # [32j+b, 32t+cc] = out[t+1, b0+b, 32j+cc]
        ob4 = ob.reshape(4, 32, NSTEP, 32)                         # (j, b, t, cc)
        out[1:, b0:b0 + B, :] = ob4.transpose(2, 1, 0, 3).reshape(NSTEP, B, N_Y)
    return out


if __name__ == "__main__":
    rng = np.random.default_rng(0)
    U = rng.standard_normal((T, BATCH, N_U)).astype(np.float32)
    Y = rng.standard_normal((PAST, BATCH, N_Y)).astype(np.float32)
    A = (rng.standard_normal((N_STATE + N_U, N_STATE)) * 0.02).astype(np.float32)
    init = rng.standard_normal((1, N_STATE)).astype(np.float32)
    o = kernel(U=U, Y=Y, A=A, init_state=init)
    print("kernel out:", o.shape, o.dtype)



# revision 6
# speedup vs baseline: 1.2888x; 1.2025x over previous
"""HCNN (known-U) recurrence kernel for 8 Trainium2 NeuronCores.

Model (see reference): 80 sequential steps of
    state' = tanh(cat(post_state, u)) @ A            A: (2112, 2048) fp32
with teacher forcing post_state[:, :128] = y during the 64 past steps,
outputs = 64 past errors then 16 forecasts (first 128 state components).

Strategy
--------
Data-parallel over batch: 256 = 8 cores x 32. Each core runs the full
recurrence for its batch slice; no collectives.

Per-core per-step matmul x @ A with batch M=32 would waste 3/4 of the
128-wide PE array, so the A columns are split into 4 interleaved groups
and computed by 4 concurrent column-tiled matmuls (tile_position=(0,32j))
sharing the array. Data is fp16 (single pass): the teacher-forced
recurrence is strongly contractive, emulation shows end-to-end output
error ~1.5e-4 relative vs the fp32 reference.

Column interleave: state column s lives in col-group j=(s//32)%4 at free
offset 32*(s//128) + s%32. With that mapping the (128, 512) psum holding
state' (batch on partitions within each 32-group) turns into the next
step's stationary operand layout via a single DVE 32x32 block-transpose:
block (j, m') lands at partitions [32j:32j+32] of k-tile m' -- exactly
where matmul round m' reads it. ACT applies tanh (psum -> fp16 SBUF),
DVE transposes, PE consumes; y/u/init contributions are pre-tanh'ed and
pre-transposed on the host, so past-step rounds k=0 (y) and k=16 (u)
have no dependency on the transpose and hide its latency.
"""

import sys

for _p in ("/opt/trn_rl_repo", "/root/.axon_site/_ro/trn_rl_repo"):
    if _p not in sys.path:
        sys.path.insert(0, _p)

import numpy as np

N_STATE = 2048
N_U = 64
N_Y = 128
PAST = 64
FORE = 16
BATCH = 256
T = PAST + FORE          # 80 total steps; only 79 matmul steps needed
NSTEP = T - 1            # step t computes state_{t+1}; state_80 is unused
NK = 17                  # contraction tiles: 16 x 128 state + 1 x (64 u + 64 pad)
KDIM = NK * 128          # 2176 padded contraction size
N_CORES = 8
B = BATCH // N_CORES     # 32 per core


def _build_program():
    import concourse.bass as bass
    import concourse.tile as tile
    from concourse import mybir

    F32 = mybir.dt.float32
    F16 = mybir.dt.float16

    nc = bass.Bass("TRN2", target_bir_lowering=False, debug=False,
                   num_devices=N_CORES)

    A_ext = nc.declare_dram_parameter("A_re", [KDIM, 4, 512], F16, isOutput=False)
    ytanhT_ext = nc.declare_dram_parameter("ytanhT", [128, PAST * B], F16, isOutput=False)
    utanhT_ext = nc.declare_dram_parameter("utanhT", [128, NSTEP * B], F16, isOutput=False)
    ywrap_ext = nc.declare_dram_parameter("ywrap", [128, (PAST - 1) * B], F32, isOutput=False)
    initxT_ext = nc.declare_dram_parameter("initxT", [128, 512], F16, isOutput=False)
    out_ext = nc.declare_dram_parameter("outbuf", [128, NSTEP * B], F32, isOutput=True)

    with tile.TileContext(nc) as tc:
        with tc.tile_pool(name="const", bufs=1) as cpool, \
             tc.tile_pool(name="xbuf", bufs=2) as xpool, \
             tc.tile_pool(name="th", bufs=2) as thpool, \
             tc.tile_pool(name="psum", bufs=2, space="PSUM") as pspool:

            # DMA order = first-use order: init x strips + y (feed step 0's
            # first rounds), then A tiles in round emission order so step 0
            # can begin as soon as the first tiles land. ywrap is only read
            # by the DVE output path, so it loads last.
            A_sb = cpool.tile([128, NK * 2048], F16, tag="A")
            ytanhT = cpool.tile([128, PAST * B], F16, tag="yt")
            utanhT = cpool.tile([128, NSTEP * B], F16, tag="ut")
            ywrap = cpool.tile([128, (PAST - 1) * B], F32, tag="yw")
            outbuf = cpool.tile([128, NSTEP * B], F32, tag="ob")
            xlo = xpool.tile([128, 256], F16, tag="xlo")
            xhi = xpool.tile([128, 256], F16, tag="xhi")

            # critical first-use loads on the sync ring; the bulk A tiles
            # split across both HWDGE rings (sync + scalar) so issue and
            # transfer bandwidth add up.
            nc.sync.dma_start(out=ytanhT[:], in_=ytanhT_ext[:])
            nc.sync.dma_start(out=A_sb[:, 0:2048], in_=A_ext[0:128, :, :])
            nc.scalar.dma_start(out=xlo[:], in_=initxT_ext[:, 0:256])
            nc.scalar.dma_start(out=xhi[:], in_=initxT_ext[:, 256:512])
            nc.sync.dma_start(out=utanhT[:], in_=utanhT_ext[:])
            nc.sync.dma_start(out=A_sb[:, 2048 * 16:2048 * 17],
                              in_=A_ext[128 * 16:128 * 17, :, :])
            korder = [0, 16] + list(range(1, 16))
            for i, k in enumerate(range(1, 16)):
                eng = nc.sync if i % 2 == 0 else nc.scalar
                eng.dma_start(out=A_sb[:, 2048 * k:2048 * (k + 1)],
                              in_=A_ext[128 * k:128 * (k + 1), :, :])
            nc.scalar.dma_start(out=ywrap[:], in_=ywrap_ext[:])

            def lhs_for(t, k, lo, hi):
                if k == 0:
                    if t < PAST:
                        return ytanhT[:, B * t:B * (t + 1)]
                    return lo[:, 0:32]
                if k == 16:
                    return utanhT[:, B * t:B * (t + 1)]
                if k < 8:
                    return lo[:, 32 * k:32 * (k + 1)]
                return hi[:, 32 * (k - 8):32 * (k - 7)]

            # Pair ks: lo+hi halves back-to-back per position so the hi MM
            # reuses the lo MM's stationary (its redundant LDWEIGHTS is
            # deleted by _dedup_ldweights — the LDW port, 4x~33ns per round,
            # is the binding resource at N=256). Tail ks: lo-only rounds
            # first so the lo psum bank closes ~0.9us before the step ends,
            # giving the tanh+transpose chain a head start; their hi halves
            # (re-loading the same weights) close the step.
            pair_ks = [0, 16] + list(range(1, 10))
            tail_ks = list(range(10, 16))
            for t in range(NSTEP):
                last = t == NSTEP - 1
                # lo = psum cols 0:256 (state cols 0:1024), hi = 256:512.
                # Separate psum tiles padded to a full 2KB bank: ACT may only
                # read one bank while the PE writes another (same-bank
                # PE-write + ACT-read is fatal and Tile serializes it).
                # The final step only needs psum cols 0:32 (the forecast).
                ps_lo = pspool.tile([128, 512], F32, tag="pslo")
                if last:
                    for idx, k in enumerate(korder):
                        lhsT = lhs_for(t, k, xlo, xhi)
                        for j in range(4):
                            nc.tensor.matmul(
                                ps_lo[32 * j:32 * (j + 1), 0:32],
                                lhsT,
                                A_sb[:, 2048 * k + 512 * j:
                                     2048 * k + 512 * j + 32],
                                start=idx == 0, stop=idx == len(korder) - 1,
                                tile_position=(0, 32 * j),
                            )
                else:
                    ps_hi = pspool.tile([128, 512], F32, tag="pshi")

                    def mm(k, j, half, start, stop):
                        ps, c0 = (ps_lo, 0) if half == 0 else (ps_hi, 256)
                        nc.tensor.matmul(
                            ps[32 * j:32 * (j + 1), 0:256],
                            lhs_for(t, k, xlo, xhi),
                            A_sb[:, 2048 * k + 512 * j + c0:
                                 2048 * k + 512 * j + c0 + 256],
                            start=start, stop=stop,
                            tile_position=(0, 32 * j),
                        )

                    for idx, k in enumerate(pair_ks):
                        for j in range(4):
                            mm(k, j, 0, idx == 0, False)
                            mm(k, j, 1, idx == 0, False)
                    for idx, k in enumerate(tail_ks):
                        for j in range(4):
                            mm(k, j, 0, False, idx == len(tail_ks) - 1)
                    for idx, k in enumerate(tail_ks):
                        for j in range(4):
                            mm(k, j, 1, False, idx == len(tail_ks) - 1)

                    th_lo = thpool.tile([128, 256], F16, tag="thlo")
                    nlo = xpool.tile([128, 256], F16, tag="xlo")
                    th_hi = thpool.tile([128, 256], F16, tag="thhi")
                    nhi = xpool.tile([128, 256], F16, tag="xhi")
                    for c0 in (0, 128):
                        nc.scalar.activation(th_lo[:, c0:c0 + 128],
                                             ps_lo[:, c0:c0 + 128],
                                             mybir.ActivationFunctionType.Tanh)
                        nc.vector.transpose(nlo[:, c0:c0 + 128],
                                            th_lo[:, c0:c0 + 128])
                    for c0 in (0, 128):
                        nc.scalar.activation(th_hi[:, c0:c0 + 128],
                                             ps_hi[:, c0:c0 + 128],
                                             mybir.ActivationFunctionType.Tanh)
                        nc.vector.transpose(nhi[:, c0:c0 + 128],
                                            th_hi[:, c0:c0 + 128])

                # output slot t+1 (expectation = cols 0:128 of state', living
                # in ps_lo[:, 0:32] across all partition groups); emitted
                # after the transposes so the DVE unblocks them first.
                if t + 1 < PAST:
                    nc.vector.tensor_sub(outbuf[:, B * t:B * (t + 1)],
                                         ps_lo[:, 0:32],
                                         ywrap[:, B * t:B * (t + 1)])
                else:
                    nc.vector.tensor_copy(outbuf[:, B * t:B * (t + 1)],
                                          ps_lo[:, 0:32])

                if not last:
                    xlo, xhi = nlo, nhi

                # stream finished output slots out while compute continues
                if t % 16 == 15:
                    nc.sync.dma_start(out=out_ext[:, B * (t - 15):B * (t + 1)],
                                      in_=outbuf[:, B * (t - 15):B * (t + 1)])

            done = (NSTEP // 16) * 16
            nc.sync.dma_start(out=out_ext[:, B * done:],
                              in_=outbuf[:, B * done:])

    _dedup_ldweights(nc)
    _split_multi_waits(nc)
    return nc


def _dedup_ldweights(nc):
    """Tile lowers each matmul into InstLdweights + InstMatmult. Our lo/hi
    psum-half pairs reload an identical stationary at the same PE tile
    position; the LDW port (one column per cycle, serialized across the four
    col positions) is the binding resource at N=256, so drop the redundant
    loads. Only loads with no sync obligations are removed."""
    removed = 0
    for f in nc.m.functions:
        for b in f.blocks:
            last = {}
            out = []
            changed = False
            for ins in b.instructions:
                tn = type(ins).__name__
                if tn == 'InstLdweights':
                    w = ins.ins[0]
                    pos = ins.tile_position
                    key = pos[1] if pos else None
                    sig = (w.memref, w.offset, str(w.ap), str(w.dtype), pos)
                    si = ins.sync_info
                    clean = si is None or (not si.on_wait and not si.on_update)
                    if last.get(key) == sig and clean:
                        removed += 1
                        changed = True
                        continue
                    last[key] = sig
                out.append(ins)
            if changed:
                b.instructions = out
    return removed


def _split_multi_waits(nc):
    """This walrus build accepts at most one sem wait per instruction; Tile
    sometimes emits more. Hoist extras onto nops inserted just before the
    instruction in the same engine stream."""
    from concourse import mybir

    n = 0
    for f in nc.m.functions:
        for b in f.blocks:
            insts = b.instructions
            out = []
            changed = False
            for ins in insts:
                si = ins.sync_info
                if si is not None and len(si.on_wait) > 1:
                    waits = list(si.on_wait)
                    for w in waits[:-1]:
                        n += 1
                        out.append(mybir.InstNoOp(
                            name=f"I-waitsplit-{n}",
                            engine=ins.engine,
                            ins=[], outs=[],
                            bass_nofuse=True,
                            sync_info=mybir.SyncInfo(on_wait=[w], on_update=[]),
                        ))
                    ins.sync_info = mybir.SyncInfo(
                        on_wait=[waits[-1]], on_update=list(si.on_update))
                    changed = True
                out.append(ins)
            if changed:
                b.instructions = out


def _host_inputs(U, Y, A, init_state):
    """Build the per-core input maps (all pre-tanh / pre-transpose work)."""
    A = np.asarray(A, np.float32)
    U = np.asarray(U, np.float32)
    Y = np.asarray(Y, np.float32)
    init_state = np.asarray(init_state, np.float32)

    A_pad = np.zeros((KDIM, N_STATE), np.float16)
    A_pad[:N_STATE + N_U] = A.astype(np.float16)
    # column interleave: col s -> (j=(s//32)%4, free 32*(s//128)+s%32)
    A_re = np.ascontiguousarray(
        A_pad.reshape(KDIM, 16, 4, 32).transpose(0, 2, 1, 3).reshape(KDIM, 4, 512))

    init_tanh = np.tanh(init_state[0]).astype(np.float16)          # (2048,)
    initxT = np.ascontiguousarray(
        np.broadcast_to(init_tanh.reshape(16, 128).T[:, None, :].transpose(0, 2, 1),
                        (128, 16, 32)).reshape(128, 512))

    ytanh = np.tanh(Y).astype(np.float16)                          # (64, 256, 128)
    utanh = np.tanh(U[:NSTEP]).astype(np.float16)                  # (79, 256, 64)

    in_maps = []
    for c in range(N_CORES):
        b0 = c * B
        yt = np.ascontiguousarray(
            ytanh[:, b0:b0 + B, :].transpose(0, 2, 1)              # (64, 128, 32)
            .transpose(1, 0, 2).reshape(128, PAST * B))
        ut = np.zeros((128, NSTEP * B), np.float16)
        ut[:N_U] = (utanh[:, b0:b0 + B, :].transpose(0, 2, 1)      # (79, 64, 32)
                    .transpose(1, 0, 2).reshape(N_U, NSTEP * B))
        # ywrap slot s (=1..63) at cols 32*(s-1): rows 32j+b = Y[s, b0+b, 32j+cc]
        yw = (Y[1:PAST, b0:b0 + B, :].reshape(PAST - 1, B, 4, 32)
              .transpose(0, 2, 1, 3)                               # (63, 4, 32b, 32cc)
              .reshape(PAST - 1, 128, 32)
              .transpose(1, 0, 2).reshape(128, (PAST - 1) * B))
        in_maps.append({
            "A_re": A_re,
            "ytanhT": yt,
            "utanhT": np.ascontiguousarray(ut),
            "ywrap": np.ascontiguousarray(yw.astype(np.float32)),
            "initxT": initxT,
        })
    return in_maps


def kernel(U, Y, A, init_state):
    from concourse.bass_utils import run_bass_kernel_spmd

    nc = _build_program()
    in_maps = _host_inputs(U, Y, A, init_state)
    res = run_bass_kernel_spmd(nc, in_maps, list(range(N_CORES)))

    out = np.empty((T, BATCH, N_Y), np.float32)
    # slot 0: err for t=0 is pure host math (state_0 = broadcast init_state)
    out[0] = np.asarray(init_state, np.float32)[0, :N_Y][None, :] - np.asarray(Y, np.float32)[0]
    for c in range(N_CORES):
        b0 = c * B
        ob = res.results[c]["outbuf"]                              # (128, 79*32)
        # [32j+b, 32t+cc] = out[t+1, b0+b, 32j+cc]
        ob4 = ob.reshape(4, 32, NSTEP, 32)                         # (j, b, t, cc)
        out[1:, b0:b0 + B, :] = ob4.transpose(2, 1, 0, 3).reshape(NSTEP, B, N_Y)
    return out


if __name__ == "__main__":
    rng = np.random.default_rng(0)
    U = rng.standard_normal((T, BATCH, N_U)).astype(np.float32)
    Y = rng.standard_normal((PAST, BATCH, N_Y)).astype(np.float32)
    A = (rng.standard_normal((N_STATE + N_U, N_STATE)) * 0.02).astype(np.float32)
    init = rng.standard_normal((1, N_STATE)).astype(np.float32)
    o = kernel(U=U, Y=Y, A=A, init_state=init)
    print("kernel out:", o.shape, o.dtype)



# revision 11
# speedup vs baseline: 1.5242x; 1.1827x over previous
"""HCNN (known-U) recurrence kernel for 8 Trainium2 NeuronCores.

Model (see reference): 80 sequential steps of
    state' = tanh(cat(post_state, u)) @ A            A: (2112, 2048) fp32
with teacher forcing post_state[:, :128] = y during the 64 past steps,
outputs = 64 past errors then 16 forecasts (first 128 state components).

Strategy
--------
Data-parallel over batch: 256 = 8 cores x 32. Each core runs the full
recurrence for its batch slice; no collectives.

Per-core per-step matmul x @ A with batch M=32 would waste 3/4 of the
128-wide PE array, so the A columns are split into 4 interleaved groups
and computed by 4 concurrent column-tiled matmuls (tile_position=(0,32j))
sharing the array. Data is fp16 (single pass): the teacher-forced
recurrence is strongly contractive, emulation shows end-to-end output
error ~1.5e-4 relative vs the fp32 reference.

Column interleave: state column s lives in col-group j=(s//32)%4 at free
offset 32*(s//128) + s%32. With that mapping the (128, 512) psum holding
state' (batch on partitions within each 32-group) turns into the next
step's stationary operand layout via a single DVE 32x32 block-transpose:
block (j, m') lands at partitions [32j:32j+32] of k-tile m' -- exactly
where matmul round m' reads it. ACT applies tanh (psum -> fp16 SBUF),
DVE transposes, PE consumes; y/u/init contributions are pre-tanh'ed and
pre-transposed on the host, so past-step rounds k=0 (y) and k=16 (u)
have no dependency on the transpose and hide its latency.
"""

import sys

for _p in ("/opt/trn_rl_repo", "/root/.axon_site/_ro/trn_rl_repo"):
    if _p not in sys.path:
        sys.path.insert(0, _p)

import numpy as np

N_STATE = 2048
N_U = 64
N_Y = 128
PAST = 64
FORE = 16
BATCH = 256
T = PAST + FORE          # 80 total steps; only 79 matmul steps needed
NSTEP = T - 1            # step t computes state_{t+1}; state_80 is unused
NK = 17                  # contraction tiles: 16 x 128 state + 1 x (64 u + 64 pad)
KDIM = NK * 128          # 2176 padded contraction size
N_CORES = 8
B = BATCH // N_CORES     # 32 per core


def _build_program():
    import concourse.bass as bass
    import concourse.tile as tile
    from concourse import mybir

    F32 = mybir.dt.float32
    F16 = mybir.dt.float16

    nc = bass.Bass("TRN2", target_bir_lowering=False, debug=False,
                   num_devices=N_CORES)

    A_ext = nc.declare_dram_parameter("A_re", [KDIM, 4, 512], F16, isOutput=False)
    ytanhT_ext = nc.declare_dram_parameter("ytanhT", [128, PAST * B], F16, isOutput=False)
    utanhT_ext = nc.declare_dram_parameter("utanhT", [128, NSTEP * B], F16, isOutput=False)
    ywrap_ext = nc.declare_dram_parameter("ywrap", [128, (PAST - 1) * B], F32, isOutput=False)
    initxT_ext = nc.declare_dram_parameter("initxT", [128, 512], F16, isOutput=False)
    out_ext = nc.declare_dram_parameter("outbuf", [128, NSTEP * B], F32, isOutput=True)

    with tile.TileContext(nc) as tc:
        with tc.tile_pool(name="const", bufs=1) as cpool, \
             tc.tile_pool(name="xbuf", bufs=2) as xpool, \
             tc.tile_pool(name="th", bufs=2) as thpool, \
             tc.tile_pool(name="psum", bufs=2, space="PSUM") as pspool:

            # DMA order = first-use order: init x strips + y (feed step 0's
            # first rounds), then A tiles in round emission order so step 0
            # can begin as soon as the first tiles land. ywrap is only read
            # by the DVE output path, so it loads last.
            A_sb = cpool.tile([128, NK * 2048], F16, tag="A")
            ytanhT = cpool.tile([128, PAST * B], F16, tag="yt")
            utanhT = cpool.tile([128, NSTEP * B], F16, tag="ut")
            ywrap = cpool.tile([128, (PAST - 1) * B], F32, tag="yw")
            outbuf = cpool.tile([128, NSTEP * B], F32, tag="ob")
            xlo = xpool.tile([128, 256], F16, tag="xlo")
            xhi = xpool.tile([128, 256], F16, tag="xhi")

            # critical first-use loads on the sync ring; the bulk A tiles
            # split across both HWDGE rings (sync + scalar) so issue and
            # transfer bandwidth add up.
            nc.sync.dma_start(out=ytanhT[:], in_=ytanhT_ext[:])
            nc.sync.dma_start(out=A_sb[:, 0:2048], in_=A_ext[0:128, :, :])
            nc.scalar.dma_start(out=xlo[:], in_=initxT_ext[:, 0:256])
            nc.scalar.dma_start(out=xhi[:], in_=initxT_ext[:, 256:512])
            nc.sync.dma_start(out=utanhT[:], in_=utanhT_ext[:])
            nc.sync.dma_start(out=A_sb[:, 2048 * 16:2048 * 17],
                              in_=A_ext[128 * 16:128 * 17, :, :])
            korder = [0, 16] + list(range(1, 16))
            for i, k in enumerate(range(1, 16)):
                eng = nc.sync if i % 2 == 0 else nc.scalar
                eng.dma_start(out=A_sb[:, 2048 * k:2048 * (k + 1)],
                              in_=A_ext[128 * k:128 * (k + 1), :, :])
            nc.scalar.dma_start(out=ywrap[:], in_=ywrap_ext[:])

            def lhs_for(t, k, lo, hi):
                if k == 0:
                    if t < PAST:
                        return ytanhT[:, B * t:B * (t + 1)]
                    return lo[:, 0:32]
                if k == 16:
                    return utanhT[:, B * t:B * (t + 1)]
                if k < 8:
                    return lo[:, 32 * k:32 * (k + 1)]
                return hi[:, 32 * (k - 8):32 * (k - 7)]

            # Pair ks: lo+hi halves back-to-back per position so the hi MM
            # reuses the lo MM's stationary (its redundant LDWEIGHTS is
            # deleted by _dedup_ldweights — the LDW port, 4x~33ns per round,
            # is the binding resource at N=256). Tail ks: lo-only rounds
            # first so the lo psum bank closes ~0.9us before the step ends,
            # giving the tanh+transpose chain a head start; their hi halves
            # (re-loading the same weights) close the step.
            pair_ks = [0, 16] + list(range(1, 10))
            tail_ks = list(range(10, 16))

            def mm(t, xl, xh, ps, k, j, half, start, stop):
                c0 = 0 if half == 0 else 256
                nc.tensor.matmul(
                    ps[32 * j:32 * (j + 1), 0:256],
                    lhs_for(t, k, xl, xh),
                    A_sb[:, 2048 * k + 512 * j + c0:
                         2048 * k + 512 * j + c0 + 256],
                    start=start, stop=stop,
                    tile_position=(0, 32 * j),
                )

            def emit_tail_hi(pend):
                # deferred hi-tail matmuls of step pt, then (only now, so
                # the writers precede the reader in emission order and Tile
                # derives reader-after-writer deps) the hi tanh+transpose
                # chain producing x_{pt+1}'s hi strips.
                pt, pxl, pxh, pps_hi, pth_hi, pnhi = pend
                for idx, k in enumerate(tail_ks):
                    for j in range(4):
                        mm(pt, pxl, pxh, pps_hi, k, j, 1,
                           False, idx == len(tail_ks) - 1)
                for c0 in (0, 128):
                    nc.scalar.activation(pth_hi[:, c0:c0 + 128],
                                         pps_hi[:, c0:c0 + 128],
                                         mybir.ActivationFunctionType.Tanh)
                    nc.vector.transpose(pnhi[:, c0:c0 + 128],
                                        pth_hi[:, c0:c0 + 128])

            # Step t's hi-tail rounds are deferred into iteration t+1,
            # emitted after t+1's dependency-free y/u pair rounds: they fill
            # the PE while step t's tanh+transpose chain produces x_{t+1},
            # so the first state round (k=1) of t+1 sees its operand ready.
            pending = None
            for t in range(NSTEP):
                last = t == NSTEP - 1
                # lo = psum cols 0:256 (state cols 0:1024), hi = 256:512.
                # Separate psum tiles padded to a full 2KB bank: ACT may only
                # read one bank while the PE writes another (same-bank
                # PE-write + ACT-read is fatal and Tile serializes it).
                # The final step only needs psum cols 0:32 (the forecast).
                ps_lo = pspool.tile([128, 512], F32, tag="pslo")
                if last:
                    if pending is not None:
                        emit_tail_hi(pending)
                        pending = None
                    for idx, k in enumerate(korder):
                        lhsT = lhs_for(t, k, xlo, xhi)
                        for j in range(4):
                            nc.tensor.matmul(
                                ps_lo[32 * j:32 * (j + 1), 0:32],
                                lhsT,
                                A_sb[:, 2048 * k + 512 * j:
                                     2048 * k + 512 * j + 32],
                                start=idx == 0, stop=idx == len(korder) - 1,
                                tile_position=(0, 32 * j),
                            )
                else:
                    ps_hi = pspool.tile([128, 512], F32, tag="pshi")
                    # hoistable head: y/u rounds (teacher-forced y only
                    # exists for past steps; the forecast k=0 round reads
                    # the recurrent state and would head-of-line-block).
                    head = [0, 16] if t < PAST else [16]
                    for idx, k in enumerate(head):
                        for j in range(4):
                            mm(t, xlo, xhi, ps_lo, k, j, 0, idx == 0, False)
                            mm(t, xlo, xhi, ps_hi, k, j, 1, idx == 0, False)
                    if pending is not None:
                        emit_tail_hi(pending)
                        pending = None
                    for k in (k for k in pair_ks if k not in head):
                        for j in range(4):
                            mm(t, xlo, xhi, ps_lo, k, j, 0, False, False)
                            mm(t, xlo, xhi, ps_hi, k, j, 1, False, False)
                    for idx, k in enumerate(tail_ks):
                        for j in range(4):
                            mm(t, xlo, xhi, ps_lo, k, j, 0,
                               False, idx == len(tail_ks) - 1)

                    th_lo = thpool.tile([128, 256], F16, tag="thlo")
                    nlo = xpool.tile([128, 256], F16, tag="xlo")
                    th_hi = thpool.tile([128, 256], F16, tag="thhi")
                    nhi = xpool.tile([128, 256], F16, tag="xhi")
                    pending = (t, xlo, xhi, ps_hi, th_hi, nhi)
                    for c0 in (0, 128):
                        nc.scalar.activation(th_lo[:, c0:c0 + 128],
                                             ps_lo[:, c0:c0 + 128],
                                             mybir.ActivationFunctionType.Tanh)
                        nc.vector.transpose(nlo[:, c0:c0 + 128],
                                            th_lo[:, c0:c0 + 128])

                # output slot t+1 (expectation = cols 0:128 of state', living
                # in ps_lo[:, 0:32] across all partition groups); emitted
                # after the transposes so the DVE unblocks them first.
                if t + 1 < PAST:
                    nc.vector.tensor_sub(outbuf[:, B * t:B * (t + 1)],
                                         ps_lo[:, 0:32],
                                         ywrap[:, B * t:B * (t + 1)])
                else:
                    nc.vector.tensor_copy(outbuf[:, B * t:B * (t + 1)],
                                          ps_lo[:, 0:32])

                if not last:
                    xlo, xhi = nlo, nhi

                # stream finished output slots out while compute continues
                if t % 16 == 15:
                    nc.sync.dma_start(out=out_ext[:, B * (t - 15):B * (t + 1)],
                                      in_=outbuf[:, B * (t - 15):B * (t + 1)])

            done = (NSTEP // 16) * 16
            nc.sync.dma_start(out=out_ext[:, B * done:],
                              in_=outbuf[:, B * done:])

    _dedup_ldweights(nc)
    _strip_dead_pe_incs(nc)
    _split_multi_waits(nc)
    return nc


def _strip_dead_pe_incs(nc):
    """Tile gives every PE instruction a counting-sem increment; with ~10k
    matmuls the EVT_SEM write unit (~26ns per inc) saturates and dependent
    engines observe psum completion ~0.5-1.0us late. Keep only increments
    whose cumulative value some wait references, and remap thresholds to
    ranks within the kept set."""
    sem = None
    # discover the PE engine counting sem name (unique per build id)
    for f in nc.m.functions:
        for b in f.blocks:
            for ins in b.instructions:
                if type(ins).__name__ != 'InstMatmult':
                    continue
                si = ins.sync_info
                for u in (si.on_update or []) if si else []:
                    if u.update_mode == 'sem-inc':
                        sem = u.ant_name
                        break
                if sem:
                    break
            if sem:
                break
        if sem:
            break
    if sem is None:
        return 0

    # collect updates (program order across blocks) and referenced values
    upds = []      # (inst, update-obj) in order
    refs = set()
    for f in nc.m.functions:
        for b in f.blocks:
            for ins in b.instructions:
                si = ins.sync_info
                if si is None:
                    continue
                for u in (si.on_update or []):
                    if u.ant_name == sem:
                        if u.update_mode != 'sem-inc' or u.update_value != 1:
                            return 0  # unexpected pattern; abort
                        upds.append((ins, u))
                for w in (si.on_wait or []):
                    if w.ant_name == sem:
                        if w.wait_mode != 'sem-ge-imm' or w.wait_value is None:
                            return 0
                        refs.add(w.wait_value)

    n = len(upds)
    keep = sorted(v for v in refs if 1 <= v <= n)
    keep_set = set(keep)
    # new threshold for wait value v = number of kept incs with index <= v
    import bisect
    from concourse import mybir as _mb

    stripped = 0
    idx_base = 0
    for f in nc.m.functions:
        for b in f.blocks:
            for ins in b.instructions:
                si = ins.sync_info
                if si is None:
                    continue
                changed = False
                new_upd = []
                for u in (si.on_update or []):
                    if u.ant_name == sem:
                        idx_base += 1
                        if idx_base not in keep_set:
                            stripped += 1
                            changed = True
                            continue
                    new_upd.append(u)
                new_wait = []
                for w in (si.on_wait or []):
                    if w.ant_name == sem:
                        w = _mb.SyncWait(
                            sync_type=w.sync_type, id=w.id,
                            ant_name=w.ant_name, wait_mode=w.wait_mode,
                            wait_value=bisect.bisect_right(keep, w.wait_value),
                            wait_reg=w.wait_reg)
                        changed = True
                    new_wait.append(w)
                if changed:
                    ins.sync_info = _mb.SyncInfo(on_wait=new_wait,
                                                 on_update=new_upd)
    return stripped


def _dedup_ldweights(nc):
    """Tile lowers each matmul into InstLdweights + InstMatmult. Our lo/hi
    psum-half pairs reload an identical stationary at the same PE tile
    position; the LDW port (one column per cycle, serialized across the four
    col positions) is the binding resource at N=256, so drop the redundant
    loads. Only loads with no sync obligations are removed."""
    removed = 0
    for f in nc.m.functions:
        for b in f.blocks:
            last = {}
            out = []
            changed = False
            for ins in b.instructions:
                tn = type(ins).__name__
                if tn == 'InstLdweights':
                    w = ins.ins[0]
                    pos = ins.tile_position
                    key = pos[1] if pos else None
                    sig = (w.memref, w.offset, str(w.ap), str(w.dtype), pos)
                    si = ins.sync_info
                    clean = si is None or (not si.on_wait and not si.on_update)
                    if last.get(key) == sig and clean:
                        removed += 1
                        changed = True
                        continue
                    last[key] = sig
                out.append(ins)
            if changed:
                b.instructions = out
    return removed


def _split_multi_waits(nc):
    """This walrus build accepts at most one sem wait per instruction; Tile
    sometimes emits more. Hoist extras onto nops inserted just before the
    instruction in the same engine stream."""
    from concourse import mybir

    n = 0
    for f in nc.m.functions:
        for b in f.blocks:
            insts = b.instructions
            out = []
            changed = False
            for ins in insts:
                si = ins.sync_info
                if si is not None and len(si.on_wait) > 1:
                    waits = list(si.on_wait)
                    for w in waits[:-1]:
                        n += 1
                        out.append(mybir.InstNoOp(
                            name=f"I-waitsplit-{n}",
                            engine=ins.engine,
                            ins=[], outs=[],
                            bass_nofuse=True,
                            sync_info=mybir.SyncInfo(on_wait=[w], on_update=[]),
                        ))
                    ins.sync_info = mybir.SyncInfo(
                        on_wait=[waits[-1]], on_update=list(si.on_update))
                    changed = True
                out.append(ins)
            if changed:
                b.instructions = out


def _host_inputs(U, Y, A, init_state):
    """Build the per-core input maps (all pre-tanh / pre-transpose work)."""
    A = np.asarray(A, np.float32)
    U = np.asarray(U, np.float32)
    Y = np.asarray(Y, np.float32)
    init_state = np.asarray(init_state, np.float32)

    A_pad = np.zeros((KDIM, N_STATE), np.float16)
    A_pad[:N_STATE + N_U] = A.astype(np.float16)
    # column interleave: col s -> (j=(s//32)%4, free 32*(s//128)+s%32)
    A_re = np.ascontiguousarray(
        A_pad.reshape(KDIM, 16, 4, 32).transpose(0, 2, 1, 3).reshape(KDIM, 4, 512))

    init_tanh = np.tanh(init_state[0]).astype(np.float16)          # (2048,)
    initxT = np.ascontiguousarray(
        np.broadcast_to(init_tanh.reshape(16, 128).T[:, None, :].transpose(0, 2, 1),
                        (128, 16, 32)).reshape(128, 512))

    ytanh = np.tanh(Y).astype(np.float16)                          # (64, 256, 128)
    utanh = np.tanh(U[:NSTEP]).astype(np.float16)                  # (79, 256, 64)

    in_maps = []
    for c in range(N_CORES):
        b0 = c * B
        yt = np.ascontiguousarray(
            ytanh[:, b0:b0 + B, :].transpose(0, 2, 1)              # (64, 128, 32)
            .transpose(1, 0, 2).reshape(128, PAST * B))
        ut = np.zeros((128, NSTEP * B), np.float16)
        ut[:N_U] = (utanh[:, b0:b0 + B, :].transpose(0, 2, 1)      # (79, 64, 32)
                    .transpose(1, 0, 2).reshape(N_U, NSTEP * B))
        # ywrap slot s (=1..63) at cols 32*(s-1): rows 32j+b = Y[s, b0+b, 32j+cc]
        yw = (Y[1:PAST, b0:b0 + B, :].reshape(PAST - 1, B, 4, 32)
              .transpose(0, 2, 1, 3)                               # (63, 4, 32b, 32cc)
              .reshape(PAST - 1, 128, 32)
              .transpose(1, 0, 2).reshape(128, (PAST - 1) * B))
        in_maps.append({
            "A_re": A_re,
            "ytanhT": yt,
            "utanhT": np.ascontiguousarray(ut),
            "ywrap": np.ascontiguousarray(yw.astype(np.float32)),
            "initxT": initxT,
        })
    return in_maps


def kernel(U, Y, A, init_state):
    from concourse.bass_utils import run_bass_kernel_spmd

    nc = _build_program()
    in_maps = _host_inputs(U, Y, A, init_state)
    res = run_bass_kernel_spmd(nc, in_maps, list(range(N_CORES)))

    out = np.empty((T, BATCH, N_Y), np.float32)
    # slot 0: err for t=0 is pure host math (state_0 = broadcast init_state)
    out[0] = np.asarray(init_state, np.float32)[0, :N_Y][None, :] - np.asarray(Y, np.float32)[0]
    for c in range(N_CORES):
        b0 = c * B
        ob = res.results[c]["outbuf"]                              # (128, 79*32)
        # [32j+b, 32t+cc] = out[t+1, b0+b, 32j+cc]
        ob4 = ob.reshape(4, 32, NSTEP, 32)                         # (j, b, t, cc)
        out[1:, b0:b0 + B, :] = ob4.transpose(2, 1, 0, 3).reshape(NSTEP, B, N_Y)
    return out


if __name__ == "__main__":
    rng = np.random.default_rng(0)
    U = rng.standard_normal((T, BATCH, N_U)).astype(np.float32)
    Y = rng.standard_normal((PAST, BATCH, N_Y)).astype(np.float32)
    A = (rng.standard_normal((N_STATE + N_U, N_STATE)) * 0.02).astype(np.float32)
    init = rng.standard_normal((1, N_STATE)).astype(np.float32)
    o = kernel(U=U, Y=Y, A=A, init_state=init)
    print("kernel out:", o.shape, o.dtype)

